# revision 24
# baseline (speedup 1.0000x reference)
"""Trainium2 Bass kernel for nn_EnhancedDLinear (8-core SPMD, full I/O).

Math reductions (vs the jax reference; verified offline, rel_err ~3e-3):
1. LayerNorm(1) degenerates -> detail_pred is the constant row dp_row
   (host-folded); the conv stack / [N,S,S] attention are dead code.
2. The k=25 replicate-pad moving average is linear (xc @ Mm); Mm folds
   into the trend/seasonal first-layer weights -> one dense [336,336] W1.
3. Channel means for the fusion MLP fold into weights (1/96) and bias.
4. Softmax left unnormalized through G/H; 1/D rides the final ReLU's
   per-partition scale operand (relu(x/D + b) == relu(x + bD)/D trick).

Sharding: N = B*C splits as one batch per core; weights replicated;
zero collectives.

Per-core kernel structure (everything bf16 except psum/scalars):
- 3 input DMAs: [wa|xb|wb] j=0 chunk first (L1 starts as soon as it
  lands), then chunks 1-2, aux weights on the scalar queue.
- Biases ride matmuls via augmented contraction rows (113th/97th/33rd/
  49th partition = 1.0 against a host-packed bias row).
- L1: 9 accumulating matmuls into one [112,288] psum, single ReLU.
- L2: 3 matmuls + 1 ones-row bias matmul -> [96, tp|sp|tsum|ssum].
- Fusion: z1/z2 matmuls; EXP row (+accum denominator) for the rank-1
  detail term, EXP cols for per-channel softmax scalars; G built with
  two fused (in0*scalar)+in1 DVE ops.
- Final: H matmul, ReLU(scale=1/D, bias=fp1b), output matmul with
  fp2b on the augmented row, copy, one store DMA.
"""

import ml_dtypes
import numpy as np

import concourse.bacc as bacc
import concourse.tile as tile
from concourse import mybir
from concourse.bass_utils import run_bass_kernel_spmd

B, S, C, P = 8, 336, 96, 96
HID = 168
MAIN_K = 25
N_CORES = 8
GRP = 626           # per-j column group in `big`: wa 336 | xb 96 | wb 194

_CACHE = {}


def _mavg_matrix(s, k):
    p = (k - 1) // 2
    m = np.zeros((s, s), np.float64)
    for j in range(s):
        for d in range(-p, p + 1):
            i = min(max(j + d, 0), s - 1)
            m[i, j] += 1.0 / k
    return m.astype(np.float32)


def _build_module():
    f32 = mybir.dt.float32
    bf16 = mybir.dt.bfloat16
    AF = mybir.ActivationFunctionType
    OP = mybir.AluOpType
    nc = bacc.Bacc("TRN2", target_bir_lowering=False, debug=False,
                   num_devices=N_CORES)

    # 4-byte-dtype, full-tensor DMAs (2-byte/sliced DMAs collapse onto one
    # SDMA engine); compute reads bf16 bitcast views.
    big0 = nc.dram_tensor("big0", [112, 216], f32, kind="ExternalInput")
    bigr = nc.dram_tensor("bigr", [112, 432], f32, kind="ExternalInput")
    aux = nc.dram_tensor("aux", [112, 735], f32, kind="ExternalInput")
    y = nc.dram_tensor("y", [P, P], f32, kind="ExternalOutput")

    with tile.TileContext(nc) as tc:
        with (
            tc.tile_pool(name="wp", bufs=1) as wp,
            tc.tile_pool(name="pp", bufs=1, space="PSUM") as pp,
        ):
            bigs0 = wp.tile([112, 216], f32, tag="bigs0")
            bigsr = wp.tile([112, 432], f32, tag="bigsr")
            auxf = wp.tile([112, 735], f32, tag="auxf")
            v0 = bigs0.bitcast(bf16)
            vr = bigsr.bitcast(bf16)
            auxs = auxf.bitcast(bf16)
            h1s = wp.tile([112, 288], bf16, tag="h1s")
            ts2 = wp.tile([97, 2], bf16, tag="ts2")
            z1s = wp.tile([33, 1], bf16, tag="z1s")
            den = wp.tile([1, 1], f32, tag="den")
            recip = wp.tile([1, 1], f32, tag="recip")
            ones48 = wp.tile([1, 48], f32, tag="ones48")
            ones96 = wp.tile([96, 1], f32, tag="ones96")
            r48 = wp.tile([48, 1], f32, tag="r48")
            fp1b = wp.tile([48, 1], f32, tag="fp1b")
            b1c = wp.tile([112, 3], f32, tag="b1c")
            ecol = wp.tile([96, 3], f32, tag="ecol")
            g0 = wp.tile([96, 96], f32, tag="g0")
            g1 = wp.tile([96, 96], f32, tag="g1")
            g2 = wp.tile([96, 96], bf16, tag="g2")
            hs = wp.tile([64, 96], bf16, tag="hs")
            out_s = wp.tile([96, 96], f32, tag="out_s")

            # constants (off critical path)
            nc.vector.memset(ts2[96:97, :], 1.0)      # ts aug row
            nc.gpsimd.memset(z1s[32:33, :], 1.0)      # fn2b aug row
            nc.gpsimd.memset(hs[32:64, :], 1.0)       # fp2b aug row (48) + junk
            nc.vector.memset(ones48, 1.0)             # f32 ones for r48 bcast
            nc.vector.memset(ones96, 1.0)             # f32 ones for den colsum

            # input DMAs: j=0 chunk first, rest behind it on the same ring
            nc.sync.dma_start(out=bigs0, in_=big0[:, :])
            nc.scalar.dma_start(out=auxf, in_=aux[:, :])
            nc.sync.dma_start(out=bigsr, in_=bigr[:, :])

            # bias columns bf16 -> f32 (ACT bias wants f32 sbuf)
            nc.gpsimd.tensor_copy(fp1b, auxs[0:48, 496:497])
            nc.gpsimd.tensor_copy(b1c, auxs[0:112, 884:887])

            # ---- L1: h1[u, c] = relu(W1.T @ xc + b1), bias on aug row ----
            ps_h1 = [pp.tile([112, 96], f32, tag=f"ps_h1{m}",
                             name=f"ps_h1{m}") for m in range(3)]
            ps_l2 = pp.tile([96, 192], f32, tag="ps_l2")
            ps_dpb = pp.tile([96, 96], f32, tag="ps_dpb")
            ps_sm = pp.tile([96, 12], f32, tag="ps_sm")

            def wsl(j, a, b):
                w = v0 if j == 0 else vr
                o = 0 if j == 0 else 432 * (j - 1)
                return w[0:112, o + a:o + b]

            def l1mm(j, m):
                nc.tensor.matmul(ps_h1[m], wsl(j, 112 * m, 112 * m + 112),
                                 wsl(j, 336, 432),
                                 start=(j == 0), stop=(j == 2))

            # j0 block of m0 first, then aux-gated rank-1 MMs fill the
            # gap while the j1/j2 chunks land, then m-major finish so each
            # m-block's relu starts as soon as its last chunk is done
            l1mm(0, 0)
            l1mm(0, 1)
            l1mm(0, 2)
            l1mm(1, 0)
            l1mm(2, 0)
            nc.scalar.activation(h1s[:, 0:96], ps_h1[0], AF.Relu,
                                 bias=b1c[:, 0:1])
            ps_ts = ps_sm[0:96, 8:10]
            nc.tensor.matmul(ps_l2, auxs[0:1, 788:884], auxs[0:1, 594:786],
                             start=True, stop=False)
            nc.tensor.matmul(ps_ts, auxs[0:1, 788:884], auxs[0:1, 786:788],
                             start=True, stop=False)
            nc.tensor.matmul(ps_dpb, auxs[0:1, 788:884], auxs[0:1, 498:594],
                             start=True, stop=True)
            for m in (1, 2):
                l1mm(1, m)
                l1mm(2, m)
                nc.scalar.activation(h1s[:, 96 * m:96 * m + 96],
                                     ps_h1[m], AF.Relu,
                                     bias=b1c[:, m:m + 1])

            # ---- L2 sum-columns first: unblocks the softmax chain ----
            for j in range(3):
                nc.tensor.matmul(
                    ps_ts, h1s[:, 96 * j:96 * j + 96],
                    auxs[0:112, 1080 + 194 * j:1082 + 194 * j],
                    start=False, stop=(j == 2))
            nc.vector.tensor_copy(ts2[0:96, :], ps_ts)
            # ---- L2 main [tp | sp] (lands in the z1/z2 shadow) ----
            for j in range(3):
                nc.tensor.matmul(
                    ps_l2, h1s[:, 96 * j:96 * j + 96],
                    auxs[0:112, 888 + 194 * j:1080 + 194 * j],
                    start=False, stop=(j == 2))

            # ---- fusion MLP: z1 = relu(fn1 @ [tmean;smean] + b1f) ----
            ps_z1 = ps_sm[0:32, 0:1]
            nc.tensor.matmul(ps_z1, auxs[0:97, 288:320], ts2[0:97, 0:1],
                             start=True, stop=False)
            nc.tensor.matmul(ps_z1, auxs[0:97, 320:352], ts2[0:97, 1:2],
                             start=False, stop=True)
            nc.vector.tensor_scalar_max(z1s[0:32, :], ps_z1, 0.0)

            # ---- z2 cols -> exp cols (unnormalized softmax scalars) ----
            ps_z2c = ps_sm[0:96, 1:4]
            for k in range(3):
                nc.tensor.matmul(ps_z2c[:, k:k + 1],
                                 auxs[0:33, 96 * k:96 * k + 96], z1s,
                                 start=True, stop=True)
            nc.scalar.activation(ecol, ps_z2c, AF.Exp)

            # ---- denominator: colsum of ecol -> reduce -> 1/D -> 48 parts ----
            ps_s3 = ps_sm[0:1, 5:8]
            nc.tensor.matmul(ps_s3, ones96, ecol, start=True, stop=True)
            ps_r48 = ps_sm[0:48, 4:5]

            # ---- G = e0*tp + e1*sp + e2*dpb (three DVE ops) ----
            nc.vector.tensor_scalar_mul(g0, ps_dpb, ecol[:, 2:3])
            nc.vector.tensor_reduce(den, ps_s3, mybir.AxisListType.X,
                                    OP.add)
            nc.vector.reciprocal(recip, den)
            nc.tensor.matmul(ps_r48, ones48, recip, start=True, stop=True)
            nc.vector.tensor_copy(r48, ps_r48)
            nc.vector.scalar_tensor_tensor(g1, ps_l2[:, 0:96],
                                           ecol[:, 0:1], g0,
                                           op0=OP.mult, op1=OP.add)
            nc.vector.scalar_tensor_tensor(g2, ps_l2[:, 96:192],
                                           ecol[:, 1:2], g1,
                                           op0=OP.mult, op1=OP.add)

            # ---- final projection ----
            ps_h = pp.tile([48, 96], f32, tag="ps_h")
            nc.tensor.matmul(ps_h, auxs[0:96, 352:400], g2,
                             start=True, stop=True)
            nc.scalar.activation(hs[0:48, :], ps_h, AF.Relu,
                                 bias=fp1b, scale=r48)
            ps_o = pp.tile([96, 96], f32, tag="ps_o")
            # lhsT = static fp2 weights (preloads during the ReLU);
            # result is out.T, untransposed on the host
            nc.tensor.matmul(ps_o, auxs[0:49, 400:496], hs[0:49, :],
                             start=True, stop=True)
            nc.vector.tensor_copy(out_s, ps_o)
            nc.sync.dma_start(out=y[:, :], in_=out_s)

    nc.compile()
    return nc


def _prep_weights(i):
    f = np.float32
    mm = _mavg_matrix(S, MAIN_K)
    w1 = np.empty((S, 2 * HID), f)
    w1[:, :HID] = mm @ i['lt1w'].T.astype(f)
    w1[:, HID:] = (np.eye(S, dtype=f) - mm) @ i['ls1w'].T.astype(f)
    b1 = np.concatenate([i['lt1b'], i['ls1b']]).astype(f)

    w2 = np.zeros((S, 194), f)
    w2[0:HID, 0:96] = i['lt2w'].T
    w2[HID:, 96:192] = i['ls2w'].T
    w2[0:HID, 192] = i['lt2w'].T.sum(1)
    w2[HID:, 193] = i['ls2w'].T.sum(1)

    # constant detail_pred row (LayerNorm(1) output == ln_b exactly)
    xf = np.full((S,), f(i['ln_b'][0]), f)
    dp_row = (np.maximum(xf @ i['op1w'].T + i['op1b'], 0)
              @ i['op2w'].T + i['op2b']).astype(f)
    dpm = dp_row.mean(dtype=f)
    b1f = (i['fn1b'] + dpm * i['fn1w'][:, 2 * C:].sum(1)).astype(f)

    big = np.zeros((112, 1296), f)
    for j in range(3):
        big[:, 432 * j:432 * j + 336] = w1[112 * j:112 * (j + 1)]

    aux = np.zeros((112, 1470), f)
    for j in range(3):
        aux[0:112, 888 + 194 * j:888 + 194 * (j + 1)] = \
            w2[112 * j:112 * (j + 1)]
    for m in range(3):
        aux[0:112, 884 + m] = b1[112 * m:112 * (m + 1)]
    aux[0:32, 0:288] = i['fn2w'].T
    aux[32, 0:288] = i['fn2b']
    aux[0, 498:594] = dp_row
    aux[0, 594:690] = i['lt2b']             # l2 bias row
    aux[0, 690:786] = i['ls2b']
    aux[0, 786] = i['lt2b'].sum()
    aux[0, 787] = i['ls2b'].sum()
    aux[0, 788:884] = 1.0                   # ones row
    aux[0:96, 288:320] = i['fn1w'][:, 0:C].T / C
    aux[0:96, 320:352] = i['fn1w'][:, C:2 * C].T / C
    aux[96, 288:320] = b1f                  # z1 bias on aug row (t block)
    aux[0:96, 352:400] = i['fp1w'].T
    aux[0:48, 400:496] = i['fp2w'].T
    aux[48, 400:496] = i['fp2b']
    aux[0:48, 496] = i['fp1b']

    return big, np.ascontiguousarray(
        aux.astype(ml_dtypes.bfloat16)).view(np.float32)


def make_in_maps(inputs):
    big, aux = _prep_weights(inputs)
    x = np.asarray(inputs['x'], np.float32)
    in_maps = []
    for b in range(N_CORES):
        bc = big.copy()
        for j in range(3):
            bc[0:112, 432 * j + 336:432 * j + 432] = \
                x[b, 112 * j:112 * (j + 1), :]
        bv = np.ascontiguousarray(bc.astype(ml_dtypes.bfloat16))
        in_maps.append(dict(
            big0=bv[:, 0:432].copy().view(np.float32),
            bigr=bv[:, 432:1296].copy().view(np.float32),
            aux=aux))
    return in_maps


def kernel(**inputs):
    if "nc" not in _CACHE:
        _CACHE["nc"] = _build_module()
    res = run_bass_kernel_spmd(_CACHE["nc"], make_in_maps(inputs),
                               core_ids=list(range(N_CORES)))
    return np.ascontiguousarray(
        np.stack([res.results[b]["y"] for b in range(N_CORES)], 0)
        .transpose(0, 2, 1))


# revision 25
# speedup vs baseline: 1.0258x; 1.0258x over previous
"""Trainium2 Bass kernel for nn_EnhancedDLinear (8-core SPMD, full I/O).

Math reductions (vs the jax reference; verified offline, rel_err ~3e-3):
1. LayerNorm(1) degenerates -> detail_pred is the constant row dp_row
   (host-folded); the conv stack / [N,S,S] attention are dead code.
2. The k=25 replicate-pad moving average is linear (xc @ Mm); Mm folds
   into the trend/seasonal first-layer weights -> one dense [336,336] W1.
3. Channel means for the fusion MLP fold into weights (1/96) and bias.
4. Softmax left unnormalized through G/H; 1/D rides the final ReLU's
   per-partition scale operand (relu(x/D + b) == relu(x + bD)/D trick).

Sharding: N = B*C splits as one batch per core; weights replicated;
zero collectives.

Per-core kernel structure (everything bf16 except psum/scalars):
- 3 input DMAs: [wa|xb|wb] j=0 chunk first (L1 starts as soon as it
  lands), then chunks 1-2, aux weights on the scalar queue.
- Biases ride matmuls via augmented contraction rows (113th/97th/33rd/
  49th partition = 1.0 against a host-packed bias row).
- L1: 9 accumulating matmuls into one [112,288] psum, single ReLU.
- L2: 3 matmuls + 1 ones-row bias matmul -> [96, tp|sp|tsum|ssum].
- Fusion: z1/z2 matmuls; EXP row (+accum denominator) for the rank-1
  detail term, EXP cols for per-channel softmax scalars; G built with
  two fused (in0*scalar)+in1 DVE ops.
- Final: H matmul, ReLU(scale=1/D, bias=fp1b), output matmul with
  fp2b on the augmented row, copy, one store DMA.
"""

import ml_dtypes
import numpy as np

import concourse.bacc as bacc
import concourse.tile as tile
from concourse import mybir
from concourse.bass_utils import run_bass_kernel_spmd

B, S, C, P = 8, 336, 96, 96
HID = 168
MAIN_K = 25
N_CORES = 8
GRP = 626           # per-j column group in `big`: wa 336 | xb 96 | wb 194

_CACHE = {}


def _mavg_matrix(s, k):
    p = (k - 1) // 2
    m = np.zeros((s, s), np.float64)
    for j in range(s):
        for d in range(-p, p + 1):
            i = min(max(j + d, 0), s - 1)
            m[i, j] += 1.0 / k
    return m.astype(np.float32)


def _build_module():
    f32 = mybir.dt.float32
    bf16 = mybir.dt.bfloat16
    AF = mybir.ActivationFunctionType
    OP = mybir.AluOpType
    nc = bacc.Bacc("TRN2", target_bir_lowering=False, debug=False,
                   num_devices=N_CORES)

    # 4-byte-dtype, full-tensor DMAs (2-byte/sliced DMAs collapse onto one
    # SDMA engine); compute reads bf16 bitcast views.
    big0 = nc.dram_tensor("big0", [112, GRP // 2], f32, kind="ExternalInput")
    bigr = nc.dram_tensor("bigr", [112, GRP], f32, kind="ExternalInput")
    aux = nc.dram_tensor("aux", [112, 444], f32, kind="ExternalInput")
    y = nc.dram_tensor("y", [P, P], f32, kind="ExternalOutput")

    with tile.TileContext(nc) as tc:
        with (
            tc.tile_pool(name="wp", bufs=1) as wp,
            tc.tile_pool(name="pp", bufs=1, space="PSUM") as pp,
        ):
            bigs0 = wp.tile([112, GRP // 2], f32, tag="bigs0")
            bigsr = wp.tile([112, GRP], f32, tag="bigsr")
            auxf = wp.tile([112, 444], f32, tag="auxf")
            v0 = bigs0.bitcast(bf16)
            vr = bigsr.bitcast(bf16)
            auxs = auxf.bitcast(bf16)
            h1s = wp.tile([112, 288], bf16, tag="h1s")
            ts2 = wp.tile([97, 2], bf16, tag="ts2")
            z1s = wp.tile([33, 1], bf16, tag="z1s")
            den = wp.tile([1, 1], f32, tag="den")
            recip = wp.tile([1, 1], f32, tag="recip")
            ones48 = wp.tile([1, 48], f32, tag="ones48")
            ones96 = wp.tile([96, 1], f32, tag="ones96")
            r48 = wp.tile([48, 1], f32, tag="r48")
            fp1b = wp.tile([48, 1], f32, tag="fp1b")
            b1c = wp.tile([112, 3], f32, tag="b1c")
            ecol = wp.tile([96, 3], f32, tag="ecol")
            g0 = wp.tile([96, 96], f32, tag="g0")
            g1 = wp.tile([96, 96], f32, tag="g1")
            g2 = wp.tile([96, 96], bf16, tag="g2")
            hs = wp.tile([64, 96], bf16, tag="hs")
            out_s = wp.tile([96, 96], f32, tag="out_s")

            # constants (off critical path)
            nc.vector.memset(ts2[96:97, :], 1.0)      # ts aug row
            nc.gpsimd.memset(z1s[32:33, :], 1.0)      # fn2b aug row
            nc.gpsimd.memset(hs[32:64, :], 1.0)       # fp2b aug row (48) + junk
            nc.vector.memset(ones48, 1.0)             # f32 ones for r48 bcast
            nc.vector.memset(ones96, 1.0)             # f32 ones for den colsum

            # input DMAs: j=0 chunk first, rest behind it on the same ring
            nc.sync.dma_start(out=bigs0, in_=big0[:, :])
            nc.scalar.dma_start(out=auxf, in_=aux[:, :])
            nc.sync.dma_start(out=bigsr, in_=bigr[:, :])

            # bias columns bf16 -> f32 (ACT bias wants f32 sbuf)
            nc.gpsimd.tensor_copy(fp1b, auxs[0:48, 496:497])
            nc.gpsimd.tensor_copy(b1c, auxs[0:112, 884:887])

            # ---- L1: h1[u, c] = relu(W1.T @ xc + b1), bias on aug row ----
            ps_h1 = [pp.tile([112, 96], f32, tag=f"ps_h1{m}",
                             name=f"ps_h1{m}") for m in range(3)]
            ps_l2 = pp.tile([96, 192], f32, tag="ps_l2")
            ps_dpb = pp.tile([96, 96], f32, tag="ps_dpb")
            ps_sm = pp.tile([96, 12], f32, tag="ps_sm")

            def wsl(j, a, b):
                w = v0 if j == 0 else vr
                o = 0 if j == 0 else GRP * (j - 1)
                return w[0:112, o + a:o + b]

            def l1mm(j, m):
                nc.tensor.matmul(ps_h1[m], wsl(j, 112 * m, 112 * m + 112),
                                 wsl(j, 336, 432),
                                 start=(j == 0), stop=(j == 2))

            # j0 block of m0 first, then aux-gated rank-1 MMs fill the
            # gap while the j1/j2 chunks land, then m-major finish so each
            # m-block's relu starts as soon as its last chunk is done
            l1mm(0, 0)
            ps_ts = ps_sm[0:96, 8:10]
            nc.tensor.matmul(ps_l2, auxs[0:1, 788:884], auxs[0:1, 594:786],
                             start=True, stop=False)
            nc.tensor.matmul(ps_ts, auxs[0:1, 788:884], auxs[0:1, 786:788],
                             start=True, stop=False)
            nc.tensor.matmul(ps_dpb, auxs[0:1, 788:884], auxs[0:1, 498:594],
                             start=True, stop=True)
            l1mm(0, 1)
            l1mm(0, 2)
            for m in range(3):
                l1mm(1, m)
                l1mm(2, m)
                nc.scalar.activation(h1s[:, 96 * m:96 * m + 96],
                                     ps_h1[m], AF.Relu,
                                     bias=b1c[:, m:m + 1])

            # ---- L2 sum-columns first: unblocks the softmax chain ----
            for j in range(3):
                nc.tensor.matmul(ps_ts, h1s[:, 96 * j:96 * j + 96],
                                 wsl(j, 624, 626),
                                 start=False, stop=(j == 2))
            nc.vector.tensor_copy(ts2[0:96, :], ps_ts)
            # ---- L2 main [tp | sp] (lands in the z1/z2 shadow) ----
            for j in range(3):
                nc.tensor.matmul(ps_l2, h1s[:, 96 * j:96 * j + 96],
                                 wsl(j, 432, 624),
                                 start=False, stop=(j == 2))

            # ---- fusion MLP: z1 = relu(fn1 @ [tmean;smean] + b1f) ----
            ps_z1 = ps_sm[0:32, 0:1]
            nc.tensor.matmul(ps_z1, auxs[0:97, 288:320], ts2[0:97, 0:1],
                             start=True, stop=False)
            nc.tensor.matmul(ps_z1, auxs[0:97, 320:352], ts2[0:97, 1:2],
                             start=False, stop=True)
            nc.vector.tensor_scalar_max(z1s[0:32, :], ps_z1, 0.0)

            # ---- z2 cols -> exp cols (unnormalized softmax scalars) ----
            ps_z2c = ps_sm[0:96, 1:4]
            for k in range(3):
                nc.tensor.matmul(ps_z2c[:, k:k + 1],
                                 auxs[0:33, 96 * k:96 * k + 96], z1s,
                                 start=True, stop=True)
            nc.scalar.activation(ecol, ps_z2c, AF.Exp)

            # ---- denominator: colsum of ecol -> reduce -> 1/D -> 48 parts ----
            ps_s3 = ps_sm[0:1, 5:8]
            nc.tensor.matmul(ps_s3, ones96, ecol, start=True, stop=True)
            ps_r48 = ps_sm[0:48, 4:5]

            # ---- G = e0*tp + e1*sp + e2*dpb (three DVE ops) ----
            nc.vector.tensor_scalar_mul(g0, ps_dpb, ecol[:, 2:3])
            nc.vector.tensor_reduce(den, ps_s3, mybir.AxisListType.X,
                                    OP.add)
            nc.vector.reciprocal(recip, den)
            nc.tensor.matmul(ps_r48, ones48, recip, start=True, stop=True)
            nc.vector.tensor_copy(r48, ps_r48)
            nc.vector.scalar_tensor_tensor(g1, ps_l2[:, 0:96],
                                           ecol[:, 0:1], g0,
                                           op0=OP.mult, op1=OP.add)
            nc.vector.scalar_tensor_tensor(g2, ps_l2[:, 96:192],
                                           ecol[:, 1:2], g1,
                                           op0=OP.mult, op1=OP.add)

            # ---- final projection ----
            ps_h = pp.tile([48, 96], f32, tag="ps_h")
            nc.tensor.matmul(ps_h, auxs[0:96, 352:400], g2,
                             start=True, stop=True)
            nc.scalar.activation(hs[0:48, :], ps_h, AF.Relu,
                                 bias=fp1b, scale=r48)
            ps_o = pp.tile([96, 96], f32, tag="ps_o")
            # lhsT = static fp2 weights (preloads during the ReLU);
            # result is out.T, untransposed on the host
            nc.tensor.matmul(ps_o, auxs[0:49, 400:496], hs[0:49, :],
                             start=True, stop=True)
            nc.vector.tensor_copy(out_s, ps_o)
            nc.sync.dma_start(out=y[:, :], in_=out_s)

    nc.compile()
    return nc


def _prep_weights(i):
    f = np.float32
    mm = _mavg_matrix(S, MAIN_K)
    w1 = np.empty((S, 2 * HID), f)
    w1[:, :HID] = mm @ i['lt1w'].T.astype(f)
    w1[:, HID:] = (np.eye(S, dtype=f) - mm) @ i['ls1w'].T.astype(f)
    b1 = np.concatenate([i['lt1b'], i['ls1b']]).astype(f)

    w2 = np.zeros((S, 194), f)
    w2[0:HID, 0:96] = i['lt2w'].T
    w2[HID:, 96:192] = i['ls2w'].T
    w2[0:HID, 192] = i['lt2w'].T.sum(1)
    w2[HID:, 193] = i['ls2w'].T.sum(1)

    # constant detail_pred row (LayerNorm(1) output == ln_b exactly)
    xf = np.full((S,), f(i['ln_b'][0]), f)
    dp_row = (np.maximum(xf @ i['op1w'].T + i['op1b'], 0)
              @ i['op2w'].T + i['op2b']).astype(f)
    dpm = dp_row.mean(dtype=f)
    b1f = (i['fn1b'] + dpm * i['fn1w'][:, 2 * C:].sum(1)).astype(f)

    big = np.zeros((112, 3 * GRP), f)
    for j in range(3):
        big[:, GRP * j:GRP * j + 336] = w1[112 * j:112 * (j + 1)]
        big[:, GRP * j + 432:GRP * j + 626] = w2[112 * j:112 * (j + 1)]

    aux = np.zeros((112, 888), f)
    for m in range(3):
        aux[0:112, 884 + m] = b1[112 * m:112 * (m + 1)]
    aux[0:32, 0:288] = i['fn2w'].T
    aux[32, 0:288] = i['fn2b']
    aux[0, 498:594] = dp_row
    aux[0, 594:690] = i['lt2b']             # l2 bias row
    aux[0, 690:786] = i['ls2b']
    aux[0, 786] = i['lt2b'].sum()
    aux[0, 787] = i['ls2b'].sum()
    aux[0, 788:884] = 1.0                   # ones row
    aux[0:96, 288:320] = i['fn1w'][:, 0:C].T / C
    aux[0:96, 320:352] = i['fn1w'][:, C:2 * C].T / C
    aux[96, 288:320] = b1f                  # z1 bias on aug row (t block)
    aux[0:96, 352:400] = i['fp1w'].T
    aux[0:48, 400:496] = i['fp2w'].T
    aux[48, 400:496] = i['fp2b']
    aux[0:48, 496] = i['fp1b']

    return big, np.ascontiguousarray(
        aux.astype(ml_dtypes.bfloat16)).view(np.float32)


def make_in_maps(inputs):
    big, aux = _prep_weights(inputs)
    x = np.asarray(inputs['x'], np.float32)
    in_maps = []
    for b in range(N_CORES):
        bc = big.copy()
        for j in range(3):
            bc[0:112, GRP * j + 336:GRP * j + 432] = \
                x[b, 112 * j:112 * (j + 1), :]
        bv = np.ascontiguousarray(bc.astype(ml_dtypes.bfloat16))
        in_maps.append(dict(
            big0=bv[:, 0:GRP].copy().view(np.float32),
            bigr=bv[:, GRP:3 * GRP].copy().view(np.float32),
            aux=aux))
    return in_maps


def kernel(**inputs):
    if "nc" not in _CACHE:
        _CACHE["nc"] = _build_module()
    res = run_bass_kernel_spmd(_CACHE["nc"], make_in_maps(inputs),
                               core_ids=list(range(N_CORES)))
    return np.ascontiguousarray(
        np.stack([res.results[b]["y"] for b in range(N_CORES)], 0)
        .transpose(0, 2, 1))
